# revision 1
# baseline (speedup 1.0000x reference)
"""Trainium2 Bass kernel for nn_Attention_87892210745803.

Full problem: x [4, 2048, 1024] fp32 -> fused QKV projection (W_qkv [3072, 1024],
b_qkv [3072]) -> 16-head causal attention (head size 64) -> out [4, 2048, 1024].

Sharding (8 cores): core c handles batch b = c // 2 and head-group g = c % 2
(8 of the 16 heads). Each core gets x[b] plus the W/b rows for its heads
(q | k | v blocks of 512 rows each) and produces out[b, :, g*512:(g+1)*512].

Per-core kernel (Bass/Tile, fp32 data, float32r matmuls — rel err ~3.4e-4):
  phase 1: PE-transpose x and W tiles (fp32 has no DMA transpose; 4 transposes
           batched per PSUM bank so each copy-back is one wide DVE op), then
           q^T/k^T with f-on-partitions (attention-ready layout) and v in
           natural layout; q/k bias added per-partition on DVE, v bias folded
           into the final output add.
  phase 2: per (i-block, head-pair): s^T = k^T q on PE — the two heads of a
           pair sit at partition bases 0/64 so their K=64 matmuls occupy
           disjoint PE row groups and overlap; exp on ACT (diagonal blocks
           shrunk to the valid >=256-wide i-window), causal zeroing via gpsimd
           affine_select, o'^T accumulated over j-tiles with a ones-column
           folded into v so the softmax denominator falls out of the same
           matmul; batched epilogue: PE transpose back, one DVE reciprocal +
           broadcast multiply per head, v-bias + store per i-block.

The two phases are software-pipelined: attention i-block I is emitted right
after QKV t-block I (it only needs qk/v of t-blocks <= I), which keeps the PE
dense through the ACT-bound exp stretches (HAM stays warm). Measured on HW:
~495 us/core/iteration (vs ~547-575 us with serial phases), rel err 3.4e-4.

Timing note: per-iteration HW time is measured in test.py by building this
kernel with an in-kernel For_i repeat loop (reps=5 vs 25) because per-dispatch
axon overhead (~13 ms) swamps the ~0.5 ms kernel.
"""

import sys

sys.path.insert(0, "/opt/trn_rl_repo")

import numpy as np

B, T, E = 4, 2048, 1024
NH_GLOBAL = 16
HS = 64
P = 128
N_CORES = 8
H = 8  # heads per core
F = H * HS  # 512: rows per q/k/v block per core

_CACHE = {}


def _build_nc(
    T=T,
    E=E,
    H=H,
    IB=512,
    use_f32r=True,
    phases=(1, 2),
    xt_bufs=1,
    stage_bufs=5,
    exp_bufs=4,
    outsb_bufs=1,
    big_bufs=4,
    ops_bufs=2,
    interleave=True,
    reps=1,
):
    import contextlib

    import concourse.bacc as bacc
    import concourse.mybir as mybir
    import concourse.tile as tile
    from concourse.masks import make_identity

    F32 = mybir.dt.float32
    OP_DT = mybir.dt.float32r if use_f32r else mybir.dt.float32
    F = H * HS
    EO = E // P  # contraction subtiles for QKV
    TT = T // P  # t-tiles
    FQK = 2 * F // P  # f-tiles for q+k
    FV_OFF = 2 * F  # v rows start in w_c
    TBS = min(512, T)  # t-block size
    NTB = T // TBS
    NI = T // IB
    JPI = IB // P

    nc = bacc.Bacc("TRN2", target_bir_lowering=False, debug=False)
    x_d = nc.dram_tensor("x", [T, E], F32, kind="ExternalInput").ap()
    w_d = nc.dram_tensor("w", [3 * F, E], F32, kind="ExternalInput").ap()
    b_d = nc.dram_tensor("b", [3 * F], F32, kind="ExternalInput").ap()
    out_d = nc.dram_tensor("out", [T, F], F32, kind="ExternalOutput").ap()

    def mm(psum, lhsT, rhs, start, stop):
        nc.tensor.matmul(psum, lhsT, rhs, start=start, stop=stop)

    with tile.TileContext(nc) as tc:
        with (
            tc.tile_pool(name="const", bufs=1) as const_pool,
            tc.tile_pool(name="persist", bufs=1) as persist,
            tc.tile_pool(name="wT", bufs=1) as wT_pool,
            tc.tile_pool(name="stage", bufs=stage_bufs) as stage,
            tc.tile_pool(name="xT", bufs=xt_bufs) as xT_pool,
            tc.tile_pool(name="exp", bufs=exp_bufs) as exp_pool,
            tc.tile_pool(name="oT", bufs=2) as oT_pool,
            tc.tile_pool(name="recip", bufs=4) as recip_pool,
            tc.tile_pool(name="outsb", bufs=outsb_bufs) as out_pool,
            tc.tile_pool(name="trp", bufs=2, space="PSUM") as trp_pool,
            tc.tile_pool(name="big", bufs=big_bufs, space="PSUM") as big_pool,
            tc.tile_pool(name="ops", bufs=ops_bufs, space="PSUM") as ops_pool,
        ):
            identity = const_pool.tile([P, P], F32)
            make_identity(nc, identity)
            b_sb = const_pool.tile([P, 3 * F // P], F32)
            nc.sync.dma_start(b_sb[:], b_d.rearrange("(o p) -> p o", p=P))
            bias_v = const_pool.tile([P, F], F32)
            nc.sync.dma_start(
                bias_v[:], b_d[None, FV_OFF : FV_OFF + F].to_broadcast((P, F))
            )

            qkT = persist.tile([P, FQK, T], OP_DT)
            v_aug = persist.tile([P, TT, H, HS + 1], OP_DT)
            ones_col = const_pool.tile([P, 1], F32)
            nc.vector.memset(ones_col, 1.0)
            # fp32r tiles need rounding producers; a converting copy qualifies
            nc.vector.tensor_copy(
                v_aug[:, :, :, HS : HS + 1],
                ones_col[:, None, None, :].to_broadcast((P, TT, H, 1)),
            )

            rep_ctx = tc.For_i(0, reps, 1) if reps > 1 else contextlib.nullcontext()
            with rep_ctx:
                # ============ phase 1: QKV projection ============
                wT = wT_pool.tile([P, EO, 3 * F], OP_DT)
                if True:
                    if 1 in phases:
                        # W^T: 4 w-row-tiles per round; their transposes share
                        # one psum bank so the copy-back is one wide DVE op
                        for wf0 in range(0, 3 * F // P, 4):
                            grp = min(4, 3 * F // P - wf0)
                            raws = []
                            for wf in range(wf0, wf0 + grp):
                                w_raw = stage.tile(
                                    [P, E], F32, tag="stage", name="w_raw"
                                )
                                half = E // 2
                                nc.sync.dma_start(
                                    w_raw[:, :half],
                                    w_d[wf * P : (wf + 1) * P, :half],
                                )
                                nc.sync.dma_start(
                                    w_raw[:, half:],
                                    w_d[wf * P : (wf + 1) * P, half:],
                                )
                                raws.append(w_raw)
                            for eo in range(EO):
                                tps = big_pool.tile(
                                    [P, 512], F32, tag="big", name="tps"
                                )
                                for k in range(grp):
                                    nc.tensor.transpose(
                                        tps[:, k * P : (k + 1) * P],
                                        raws[k][:, eo * P : (eo + 1) * P],
                                        identity,
                                    )
                                nc.vector.tensor_copy(
                                    wT[:, eo, wf0 * P : (wf0 + grp) * P],
                                    tps[:, : grp * P],
                                )

                        def p1_tblock(tb):
                            xT = xT_pool.tile(
                                [P, EO, TBS], OP_DT, tag="xT", name="xT"
                            )
                            raws = []
                            for tt in range(TBS // P):
                                git = tb * (TBS // P) + tt
                                x_raw = stage.tile(
                                    [P, E], F32, tag="stage", name="x_raw"
                                )
                                half = E // 2
                                nc.sync.dma_start(
                                    x_raw[:, :half],
                                    x_d[git * P : (git + 1) * P, :half],
                                )
                                nc.sync.dma_start(
                                    x_raw[:, half:],
                                    x_d[git * P : (git + 1) * P, half:],
                                )
                                raws.append(x_raw)
                            for eo in range(EO):
                                tps = big_pool.tile(
                                    [P, 512], F32, tag="big", name="tps"
                                )
                                for k in range(TBS // P):
                                    nc.tensor.transpose(
                                        tps[:, k * P : (k + 1) * P],
                                        raws[k][:, eo * P : (eo + 1) * P],
                                        identity,
                                    )
                                nc.vector.tensor_copy(xT[:, eo, :], tps[:, :TBS])

                            # q^T / k^T tiles: psum[f=128, t=TBS]
                            for wf in range(FQK):
                                ps = big_pool.tile(
                                    [P, 512], F32, tag="big", name="qkps"
                                )[:, :TBS]
                                for eo in range(EO):
                                    mm(
                                        ps,
                                        wT[:, eo, wf * P : (wf + 1) * P],
                                        xT[:, eo, :],
                                        start=(eo == 0),
                                        stop=(eo == EO - 1),
                                    )
                                nc.vector.tensor_scalar_add(
                                    qkT[:, wf, tb * TBS : (tb + 1) * TBS],
                                    ps,
                                    b_sb[:, wf : wf + 1],
                                )

                            # v tiles: psum[t=128, f=F]; bias folded in at end
                            for tt in range(TBS // P):
                                git = tb * (TBS // P) + tt
                                ps = big_pool.tile(
                                    [P, 512], F32, tag="big", name="vps"
                                )[:, :F]
                                for eo in range(EO):
                                    mm(
                                        ps,
                                        xT[:, eo, tt * P : (tt + 1) * P],
                                        wT[:, eo, FV_OFF : FV_OFF + F],
                                        start=(eo == 0),
                                        stop=(eo == EO - 1),
                                    )
                                nc.vector.tensor_copy(
                                    v_aug[:, git, :, 0:HS],
                                    ps.rearrange("p (h d) -> p h d", d=HS),
                                )
                    else:
                        nc.vector.memset(qkT[:].bitcast(F32), 0.0)
                        nc.vector.memset(v_aug[:, :, :, 0:HS].bitcast(F32), 0.0)

                # ============ phase 2: attention ============
                def head_epilogue(ops_t, out_sb, h):
                    nit = IB // P
                    oT = oT_pool.tile([P, IB], F32, tag="oT", name="oT")
                    nc.vector.tensor_copy(oT[: HS + 1, :], ops_t[: HS + 1, :])
                    tp = trp_pool.tile([P, nit, HS + 1], F32, tag="tr", name="tp")
                    for it in range(nit):
                        nc.tensor.transpose(
                            tp[:, it, :],
                            oT[: HS + 1, it * P : (it + 1) * P],
                            identity[: HS + 1, : HS + 1],
                        )
                    oTT = oT_pool.tile([P, nit, HS + 1], F32, tag="oTT", name="oTT")
                    nc.vector.tensor_copy(oTT, tp)
                    rc = recip_pool.tile([P, nit], F32, tag="recip", name="rc")
                    nc.vector.reciprocal(rc, oTT[:, :, HS])
                    nc.vector.tensor_tensor(
                        out_sb[:, :, h * HS : (h + 1) * HS],
                        oTT[:, :, 0:HS],
                        rc[:, :, None].to_broadcast((P, nit, HS)),
                        mybir.AluOpType.mult,
                    )

                if 2 in phases:
                    if True:
                        def p2_iblock(I):
                            out_sb = out_pool.tile(
                                [P, IB // P, F], F32, tag="outsb", name="out_sb"
                            )
                            njt = JPI * (I + 1)
                            for hp in range(H // 2):
                                # head pair at partition bases 0/64 of one
                                # f-tile: disjoint PE row groups, s^T matmuls
                                # overlap in hardware
                                fq = hp
                                fk = F // P + hp
                                ops_pair = [
                                    ops_pool.tile(
                                        [P, 512], F32, tag="ops", name="ops_t"
                                    )[:, :IB]
                                    for _ in range(2)
                                ]
                                for jt in range(njt):
                                    r = jt - JPI * I
                                    # diagonal blocks: shrink to the valid
                                    # i-window (>=256 wide for fp32r full rate)
                                    off = 0 if r < 0 else min(P * r, IB - 256)
                                    w = IB - off
                                    exs = []
                                    for half in range(2):
                                        pb = half * HS
                                        sp = big_pool.tile(
                                            [P, 512], F32, tag="big", name="sp"
                                        )[:, :w]
                                        mm(
                                            sp,
                                            qkT[
                                                pb : pb + HS,
                                                fk,
                                                jt * P : (jt + 1) * P,
                                            ],
                                            qkT[
                                                pb : pb + HS,
                                                fq,
                                                I * IB + off : (I + 1) * IB,
                                            ],
                                            start=True,
                                            stop=True,
                                        )
                                        ex = exp_pool.tile(
                                            [P, IB], OP_DT, tag="exp", name="ex"
                                        )[:, :w]
                                        nc.scalar.activation(
                                            ex,
                                            sp,
                                            mybir.ActivationFunctionType.Exp,
                                            scale=0.125,
                                        )
                                        if r >= 0:
                                            # causal: keep where i >= j
                                            nc.gpsimd.affine_select(
                                                out=ex,
                                                in_=ex,
                                                compare_op=mybir.AluOpType.is_ge,
                                                fill=0.0,
                                                base=off - P * r,
                                                channel_multiplier=-1,
                                                pattern=[[1, w]],
                                            )
                                        exs.append(ex)
                                    for half in range(2):
                                        mm(
                                            ops_pair[half][: HS + 1, off:],
                                            v_aug[:, jt, 2 * hp + half, :],
                                            exs[half],
                                            start=(jt == 0),
                                            stop=(jt == njt - 1),
                                        )
                                head_epilogue(ops_pair[0], out_sb, 2 * hp)
                                head_epilogue(ops_pair[1], out_sb, 2 * hp + 1)
                            for it in range(IB // P):
                                git = I * (IB // P) + it
                                nc.vector.tensor_add(
                                    out=out_sb[:, it, :],
                                    in0=out_sb[:, it, :],
                                    in1=bias_v,
                                )
                                nc.sync.dma_start(
                                    out_d[git * P : (git + 1) * P, :],
                                    out_sb[:, it, :],
                                )
                # ---- driver: software-pipeline QKV t-blocks with attention
                # i-blocks (attention for i-block I only needs qk/v of
                # t-blocks <= I) so PE stays dense while ACT grinds exp ----
                if 1 in phases:
                    for _t in range(NTB):
                        p1_tblock(_t)
                        if 2 in phases and interleave:
                            p2_iblock(_t)
                if 2 in phases and not (1 in phases and interleave):
                    for _i in range(NI):
                        p2_iblock(_i)
                if 2 not in phases:
                    for git in range(TT):
                        nc.sync.dma_start(
                            out_d[git * P : (git + 1) * P, :],
                            qkT[
                                :, git // 4, (git % 4) * 512 : (git % 4 + 1) * 512
                            ].bitcast(F32),
                        )

    nc.compile()
    return nc


def get_nc():
    if "nc" not in _CACHE:
        _CACHE["nc"] = _build_nc()
    return _CACHE["nc"]


def shard_inputs(x, W_qkv, b_qkv):
    """Split full inputs into the 8 per-core input maps."""
    in_maps = []
    for c in range(N_CORES):
        b_, g = c // 2, c % 2
        rq = slice(g * F, (g + 1) * F)
        rk = slice(E + g * F, E + (g + 1) * F)
        rv = slice(2 * E + g * F, 2 * E + (g + 1) * F)
        w_c = np.concatenate([W_qkv[rq], W_qkv[rk], W_qkv[rv]], axis=0)
        b_c = np.concatenate([b_qkv[rq], b_qkv[rk], b_qkv[rv]], axis=0)
        in_maps.append(
            {
                "x": np.ascontiguousarray(x[b_], dtype=np.float32),
                "w": np.ascontiguousarray(w_c, dtype=np.float32),
                "b": np.ascontiguousarray(b_c, dtype=np.float32),
            }
        )
    return in_maps


def gather_output(results):
    """Assemble per-core [T, F] outputs into the full [B, T, E] output."""
    out = np.empty((B, T, E), dtype=np.float32)
    for c in range(N_CORES):
        b_, g = c // 2, c % 2
        out[b_, :, g * F : (g + 1) * F] = results[c]["out"]
    return out


def kernel(x, W_qkv, b_qkv):
    from concourse.bass_utils import run_bass_kernel_spmd

    x = np.asarray(x, dtype=np.float32)
    W_qkv = np.asarray(W_qkv, dtype=np.float32)
    b_qkv = np.asarray(b_qkv, dtype=np.float32)
    in_maps = shard_inputs(x, W_qkv, b_qkv)
    res = run_bass_kernel_spmd(get_nc(), in_maps, core_ids=list(range(N_CORES)))
    return gather_output(res.results)

